# revision 35
# baseline (speedup 1.0000x reference)
"""ChannelBlock (XCiT-style cross-covariance attention + MLP w/ ECA gate) on 8 TRN2 cores.

Sharding: data-parallel over batch B=8 (1 batch element per core); all params
replicated.  Per-core problem: x (4096, 512) fp32.  HW ~370-374us (baseline 514us).

Design notes:
  - attn-apply folded into proj: per head G_h[e,c] = sum_d attn_h[d,e]*projW[c,64h+d],
    so x2 = q @ G in one fp8-DoubleRow GEMM chain (no separate apply pass).
  - phase-2/3 token chunks are PERMUTED (chunk a = tokens {8i+a}): the
    reference's .view(B,C,H,W) channel shuffle maps fc2's channel-major output
    tile for chunk a DIRECTLY onto output rows [512a, 512a+512) -- no permute
    pass.  qT is written in permuted order during phase 1 (strided ACT copy).
  - fp8-e4m3 DoubleRow (2 contraction rows/cycle) on fc2 (gelu emits fp8 h1T;
    weights host-quantized at 8x, undone by the y-copy ACT scale), on the
    x2 = q @ G GEMM, and on HALF of fc1's contraction (channels 0-255 as one
    DR-MM + channels 256-511 as two bf16 MMs: error scales with sqrt of the
    quantized fraction).  qkv/logits and the other fc1 half stay bf16:
    measured rel_l2 1.771e-2 vs the 2e-2 gate (full-fc1 fp8 would be 2.03e-2).
  - transposes via NORMAL matmul with identity as the moving operand
    (out = curs.T @ I): pipelines at ~90ns vs ~390ns for transpose-mode.
  - ECA pool via ones-vector matmuls on fc2 output tiles (PE, partition
    reduction); for the LAST chunk pooled comes from h1 @ colsum(fc2_w) right
    after gelu, so the gate chain overlaps the final fc2/y work.
  - out_d (bf16) doubles as the x2 scratch: phase-2/3 writes x2 rows strided;
    the tail adds gated y via two parallel streams (gpsimd accum-DMA for
    blocks 0-3, sync-queue readback + DVE add for 4-7).  Host casts to f32.
  - startup: x chunk 0 split across sy/s queues, chunk 1 across s/g, LN1
    pipelined 2 chunks ahead of the PE; phase-2/3 weights prefetched at nt==2
    on sy/g so the ACT queue keeps feeding PSUM-evacuation copies.
"""

import numpy as np
import ml_dtypes
from contextlib import ExitStack

import concourse.bacc as bacc
import concourse.bass as bass
import concourse.mybir as mybir
import concourse.tile as tile
from concourse.bass import ts, ds
from concourse.bass_utils import run_bass_kernel_spmd
from concourse.masks import make_identity

F32 = mybir.dt.float32
BF16 = mybir.dt.bfloat16
F16 = mybir.dt.float16
FP8 = mybir.dt.float8e4
DR = mybir.MatmulPerfMode.DoubleRow
AF = mybir.ActivationFunctionType
ALU = mybir.AluOpType
AX = mybir.AxisListType

B = 8
NTOK = 4096
C = 512
NH = 8
HD = 64
HID = 2048
NT = 8           # token chunks of 512
TCH = NTOK // NT  # 512 tokens per chunk
P = 128
LN_EPS = 1e-5
SCALE = HD ** -0.5
FC2_WSCALE = 8.0   # fc2 weights stored *8 in fp8; undone by ACT scale
FC2_FP8 = True


def _build(flags):
    """Build the per-core bass program. flags: dict of adaptive bools."""
    nc = bacc.Bacc("TRN2", target_bir_lowering=False, debug=False, num_devices=B)

    x_d = nc.dram_tensor("x", (NTOK, C), F32, kind="ExternalInput").ap()
    wqkvT_d = nc.dram_tensor("wqkvT", (C, 3 * C), BF16, kind="ExternalInput").ap()
    projwT_d = nc.dram_tensor("projwT", (C, C), BF16, kind="ExternalInput").ap()
    fc1wT_d = nc.dram_tensor("fc1wT", (C, HID), BF16, kind="ExternalInput").ap()
    fc1w8T_d = nc.dram_tensor("fc1w8T", (2 * P, HID), FP8, kind="ExternalInput").ap()
    if FC2_FP8:
        fc2wT_d = nc.dram_tensor("fc2wT", (HID, C), FP8, kind="ExternalInput").ap()
    else:
        fc2wT_d = nc.dram_tensor("fc2wT", (HID, C), BF16, kind="ExternalInput").ap()
    fc1b_d = nc.dram_tensor("fc1b", (P, HID // P), F32, kind="ExternalInput").ap()
    fc2b_d = nc.dram_tensor("fc2b", (P, C // P), F32, kind="ExternalInput").ap()
    ecaw_d = nc.dram_tensor("ecaw", (1, 3), F32, kind="ExternalInput").ap()
    w2s_d = nc.dram_tensor("w2s", (P, HID // P), FP8 if FC2_FP8 else BF16,
                           kind="ExternalInput").ap()
    fc2bsn_d = nc.dram_tensor("fc2bsn", (1, 1), F32, kind="ExternalInput").ap()
    if flags["proj_bias"]:
        projb_d = nc.dram_tensor("projb", (1, C), BF16, kind="ExternalInput").ap()
    ln_d = {}
    for nm in ("ln1w", "ln1b", "ln2w", "ln2b"):
        if flags[nm]:
            ln_d[nm] = nc.dram_tensor(nm, (C,), F32, kind="ExternalInput").ap()

    out_d = nc.dram_tensor("out", (NTOK, C), BF16, kind="ExternalOutput").ap()

    v = nc.vector
    g = nc.gpsimd
    s = nc.scalar
    t = nc.tensor
    sy = nc.sync

    # strided views: token n = 1024*j + 8*p + a  <->  [a][p, j, :]
    x_perm = x_d.rearrange("(j p e) c -> e p j c", e=8, p=P)
    x2_perm = out_d.rearrange("(j p e) c -> e p j c", e=8, p=P)

    with tile.TileContext(nc) as tc, ExitStack() as ctx:
        # ---------------- pools ----------------
        consts = ctx.enter_context(tc.tile_pool(name="consts", bufs=1))
        wpool = ctx.enter_context(tc.tile_pool(name="wpool", bufs=1))
        qpool = ctx.enter_context(tc.tile_pool(name="qpool", bufs=1))
        xin = ctx.enter_context(tc.tile_pool(name="xin", bufs=4))
        curp = ctx.enter_context(tc.tile_pool(name="curp", bufs=5))
        curTp = ctx.enter_context(tc.tile_pool(name="curTp", bufs=2))
        kvp = ctx.enter_context(tc.tile_pool(name="kvp", bufs=4))
        statp = ctx.enter_context(tc.tile_pool(name="statp", bufs=3))
        smp = ctx.enter_context(tc.tile_pool(name="smp", bufs=1))
        x2p = ctx.enter_context(tc.tile_pool(name="x2p", bufs=4))
        h1p = ctx.enter_context(tc.tile_pool(name="h1p", bufs=1))
        yp = ctx.enter_context(tc.tile_pool(name="yp", bufs=1))
        outp = ctx.enter_context(tc.tile_pool(name="outp", bufs=2))

        ps_t = ctx.enter_context(tc.tile_pool(name="ps_t", bufs=2, space="PSUM"))
        ps_mm = ctx.enter_context(tc.tile_pool(name="ps_mm", bufs=3, space="PSUM"))
        ps_log = ctx.enter_context(tc.tile_pool(name="ps_log", bufs=1, space="PSUM"))
        ps_pool = ctx.enter_context(tc.tile_pool(name="ps_pool", bufs=1, space="PSUM"))

        # ---------------- phase-1 critical DMAs first ----------------
        # x chunk 0 split into 4 so LN1 starts on first-quarter arrival;
        # chunks 1-2 prefetched immediately after.  Weights for phase 1
        # (wqkv) ride the scalar queue in parallel; everything else waits.
        xbs = {}
        xbs[0] = xin.tile([P, 4, C], F32, name="xb0", tag="xb", bufs=3)
        for q4 in range(4):
            (sy if q4 < 2 else s).dma_start(out=xbs[0][:, q4, :],
                                            in_=x_d[ds(q4 * P, P), :])
        wqkv_sb = wpool.tile([P, 4, 3 * C], BF16)
        for cj in range(4):
            (s if cj % 2 else g).dma_start(out=wqkv_sb[:, cj, :],
                                           in_=wqkvT_d[ts(cj, P), :])
        ident = consts.tile([P, P], BF16)
        make_identity(nc, ident)
        ones_colh = consts.tile([P, 1], F16)   # lhsT for partition-sum of y tiles
        v.memset(ones_colh, 1.0)
        ones_row = consts.tile([1, P], BF16)   # lhsT for broadcast outer product
        v.memset(ones_row, 1.0)
        if flags["proj_bias"]:
            ones_row_bf = consts.tile([1, P], BF16)
            v.memset(ones_row_bf, 1.0)
        # preload the Exp act-table off the critical path (softmax boundary)
        tiny = consts.tile([1, 1], F32)
        v.memset(tiny, 0.0)
        s.activation(out=tiny, in_=tiny, func=AF.Exp)

        ln_bc = {}
        for nm in ln_d:
            bc = wpool.tile([P, C], F32, tag=f"lnbc_{nm}")
            g.dma_start(
                out=bc,
                in_=bass.AP(tensor=ln_d[nm].tensor, offset=ln_d[nm].offset,
                            ap=[[0, P], [1, C]]),
            )
            ln_bc[nm] = bc

        qT_sb = qpool.tile([P, 4, NTOK], FP8)

        def ln_dve(src_tiles, w_bc, b_bc, apply_eng="v"):
            """LayerNorm stats+apply on DVE only -> 4 bf16 cur tiles."""
            mv = statp.tile([P, 4, 2], F32, tag="mv")
            st = statp.tile([P, 6], F32, tag="st6")
            for p in range(4):
                v.bn_stats(out=st, in_=src_tiles[p])
                v.bn_aggr(out=mv[:, p, :], in_=st)
                st = statp.tile([P, 6], F32, tag="st6")
            # rstd = 1/sqrt(var+eps) via DVE reciprocal + 3 Newton steps
            # (avoids ScalarE Sqrt table load between Exp/Gelu phases)
            aN = statp.tile([P, 4], F32, tag="veps")
            v.tensor_scalar_add(out=aN, in0=mv[:, :, 1], scalar1=LN_EPS)
            rstd = statp.tile([P, 4], F32, tag="rstd")
            v.reciprocal(out=rstd, in_=aN)
            tN = statp.tile([P, 4], F32, tag="tN")
            uN = statp.tile([P, 4], F32, tag="uN")
            for _ in range(2):
                v.tensor_mul(out=tN, in0=rstd, in1=rstd)
                v.tensor_mul(out=tN, in0=tN, in1=aN)
                v.tensor_scalar(out=uN, in0=tN, scalar1=-0.5, scalar2=1.5,
                                op0=ALU.mult, op1=ALU.add)
                v.tensor_mul(out=rstd, in0=rstd, in1=uN)
            nmr = None
            if apply_eng == "s":
                nmr = statp.tile([P, 4], F32, tag="nmr")
                v.tensor_mul(out=nmr, in0=mv[:, :, 0], in1=rstd)
                v.tensor_scalar_mul(out=nmr, in0=nmr, scalar1=-1.0)
            curs = []
            for p in range(4):
                if w_bc is None and b_bc is None:
                    cur = curp.tile([P, TCH], BF16, tag="cur", bufs=12)
                    if apply_eng == "s":
                        s.activation(out=cur, in_=src_tiles[p], func=AF.Identity,
                                     bias=nmr[:, p:p + 1], scale=rstd[:, p:p + 1])
                    else:
                        v.tensor_scalar(out=cur, in0=src_tiles[p],
                                        scalar1=mv[:, p, 0:1], scalar2=rstd[:, p:p + 1],
                                        op0=ALU.subtract, op1=ALU.mult)
                else:
                    tmp = curp.tile([P, TCH], F32, tag="curf")
                    v.tensor_scalar(out=tmp, in0=src_tiles[p],
                                    scalar1=mv[:, p, 0:1], scalar2=rstd[:, p:p + 1],
                                    op0=ALU.subtract, op1=ALU.mult)
                    cur = curp.tile([P, TCH], BF16, tag="cur", bufs=12)
                    if w_bc is not None and b_bc is not None:
                        v.tensor_mul(out=tmp, in0=tmp, in1=w_bc)
                        v.tensor_add(out=cur, in0=tmp, in1=b_bc)
                    elif w_bc is not None:
                        v.tensor_mul(out=cur, in0=tmp, in1=w_bc)
                    else:
                        v.tensor_add(out=cur, in0=tmp, in1=b_bc)
                curs.append(cur)
            return curs

        def ln_pe(curs, want_fp8=False):
            """Transpose 4 cur tiles -> channel-major curT [128,4,512].

            Uses normal matmuls with identity as the MOVING operand
            (out = curs_slice.T @ I): pipelines at ~90ns/MM vs ~390ns
            for transpose-mode (which can't overlap fill/drain)."""
            curT = curTp.tile([P, 4, TCH], BF16, tag="curT")
            curT8 = None
            if want_fp8:
                curT8 = curTp.tile([P, 2, TCH], FP8, tag="curT8")
            for cj in range(4):
                pst = ps_t.tile([P, TCH], F32, tag="pst")
                for p in range(4):
                    t.matmul(pst[:, ts(p, P)], lhsT=curs[p][:, ts(cj, P)],
                             rhs=ident, start=True, stop=True)
                s.copy(out=curT[:, cj, :], in_=pst)
                if want_fp8 and cj < 2:
                    v.tensor_copy(out=curT8[:, cj, :], in_=pst)
            return (curT, curT8) if want_fp8 else curT

        # ================= PHASE 1: LN1 + qkv + logits (sw-pipelined) ======
        logits_ps = ps_log.tile([P, 4, P], F32)
        qT_v = qT_sb.rearrange("p jc (a i) -> p jc a i", a=8)

        def pe_block1(nt, curs):
            curT = ln_pe(curs)
            kvts = []
            for p in range(4):
                kvt = kvp.tile([P, 2 * C], BF16, tag="kv")
                for h2 in range(2):
                    ps = ps_mm.tile([P, TCH], F32, tag="mm")
                    for cj in range(4):
                        t.matmul(ps, lhsT=curT[:, cj, ts(p, P)],
                                 rhs=wqkv_sb[:, cj, ds(C + h2 * C, C)],
                                 start=(cj == 0), stop=(cj == 3))
                    (v.tensor_copy if h2 == 0 else s.copy)(
                        out=kvt[:, ts(h2, C)], in_=ps)
                kvts.append(kvt)

            def do_q():
                for jc in range(4):
                    ps = ps_mm.tile([P, TCH], F32, tag="mm")
                    for cj in range(4):
                        t.matmul(ps, lhsT=wqkv_sb[:, cj, ts(jc, P)],
                                 rhs=curT[:, cj, :],
                                 start=(cj == 0), stop=(cj == 3))
                    # permuted write: qT[:, jc, a*512 + 64*nt + i'] = q[8i'+a]
                    s.copy(out=qT_v[:, jc, :, ds(64 * nt, 64)],
                           in_=ps.rearrange("p (i a) -> p a i", a=8))

            def do_logits():
                for p in range(4):
                    for hp in range(4):
                        t.matmul(logits_ps[:, hp, :],
                                 lhsT=kvts[p][:, ds(hp * P, P)],
                                 rhs=kvts[p][:, ds(C + hp * P, P)],
                                 start=(nt == 0 and p == 0 and hp == 0),
                                 stop=(nt == NT - 1 and p == 3 and hp == 3),
                                 skip_group_check=True)

            if nt == NT - 1:
                do_logits()
                do_q()
            else:
                do_q()
                do_logits()

        pends = []
        for nt in range(NT):
            xb = xbs.pop(nt)
            xts = [xb[:, q, :] for q in range(4)]
            curs = ln_dve(xts, ln_bc.get("ln1w"), ln_bc.get("ln1b"))
            # prefetch AFTER ln_dve so the scheduler orders this chunk's
            # LN apply ahead of the next chunks' DMA-dependent stats
            pres = (1, 2, 3) if nt == 0 else ((nt + 3,) if nt + 3 < NT else ())
            for pre in pres:
                xbs[pre] = xin.tile([P, 4, C], F32, name=f"xb{nt}_{pre}", tag="xb", bufs=3)
                if pre == 1:
                    for q4 in range(4):
                        (s if q4 % 2 else g).dma_start(
                            out=xbs[pre][:, q4, :],
                            in_=x_d[ds(pre * TCH + q4 * P, P), :])
                else:
                    sy.dma_start(
                        out=xbs[pre],
                        in_=x_d[ds(pre * TCH, TCH), :].rearrange(
                            "(q p) c -> p q c", p=P))
            pends.append((nt, curs))
            if len(pends) > 2:
                pe_block1(*pends.pop(0))
            if nt == 2:
                # prefetch phase-2/3 weights (sync/gpsimd queues: keeps the
                # ACT queue free for the copy stream)
                projw_sb = wpool.tile([P, 4, C], BF16)
                for dc in range(4):
                    sy.dma_start(out=projw_sb[:, dc, :], in_=projwT_d[ts(dc, P), :])
                fc1w_sb = wpool.tile([P, 4, HID], BF16)
                for cj in range(4):
                    sy.dma_start(out=fc1w_sb[:, cj, :], in_=fc1wT_d[ts(cj, P), :])
                fc1w8_sb = wpool.tile([P, 2, HID], FP8)
                sy.dma_start(out=fc1w8_sb,
                             in_=fc1w8T_d[:, :].rearrange("(q p) c -> p q c", p=P))
                fc2w_sb = wpool.tile([P, 16, C], FP8 if FC2_FP8 else BF16)
                for jc in range(4):
                    g.dma_start(out=fc2w_sb[:, ts(jc, 4), :],
                                in_=fc2wT_d[ds(jc * 4 * P, 4 * P), :].rearrange(
                                    "(q p) c -> p q c", p=P))
                fc1b_sb = wpool.tile([P, HID // P], F32)
                sy.dma_start(out=fc1b_sb, in_=fc1b_d[:, :])
                fc2b_sb = wpool.tile([P, C // P], F32)
                sy.dma_start(out=fc2b_sb, in_=fc2b_d[:, :])
                eca_sb = wpool.tile([1, 3], F32)
                g.dma_start(out=eca_sb, in_=ecaw_d[:, :])
                w2s_sb = wpool.tile([P, HID // P], FP8 if FC2_FP8 else BF16)
                g.dma_start(out=w2s_sb, in_=w2s_d[:, :])
                fc2bsn_sb = wpool.tile([1, 1], F32)
                g.dma_start(out=fc2bsn_sb, in_=fc2bsn_d[:, :])
                if flags["proj_bias"]:
                    projb_sb = wpool.tile([1, C], BF16)
                    g.dma_start(out=projb_sb, in_=projb_d[:, :])
        for pnd in pends:
            pe_block1(*pnd)

        # ================= softmax + G = blockdiag(attn)-contracted projW ====
        G_sb = wpool.tile([P, 4, C], FP8)
        for hp in range(4):
            a128 = smp.tile([P, P], BF16, tag="a128", bufs=2)
            for half in range(2):
                rows = slice(64 * half, 64 * half + 64)
                nm = smp.tile([P, 1], F32, tag="nm", bufs=2)
                v.tensor_reduce(out=nm[rows, :], in_=logits_ps[rows, hp, ds(64 * half, 64)],
                                axis=AX.X, op=ALU.max, negate=True)
                esb = smp.tile([P, 64], F32, tag="esb", bufs=2)
                ssum = smp.tile([P, 1], F32, tag="ssum", bufs=2)
                s.activation(out=esb[rows, :], in_=logits_ps[rows, hp, ds(64 * half, 64)],
                             func=AF.Exp, bias=nm[rows, :], scale=1.0,
                             accum_out=ssum[rows, :])
                v.reciprocal(out=ssum[rows, :], in_=ssum[rows, :])
                v.tensor_scalar_mul(out=a128[rows, ds(64 * half, 64)],
                                    in0=esb[rows, :], scalar1=ssum[rows, :])
            # G_h[e, c] = sum_d attn_h[d, e] * projwT[64h+d, c]
            gps = ps_mm.tile([P, C], F32, tag="mm")
            for half in range(2):
                rows = slice(64 * half, 64 * half + 64)
                t.matmul(gps[rows, :], lhsT=a128[rows, rows],
                         rhs=projw_sb[rows, hp, :], start=True, stop=True)
            s.copy(out=G_sb[:, hp, :], in_=gps)

        # ================= PHASE 2+3: x2 = q@G (+x), LN2, MLP (permuted) =====
        pool_ps = ps_pool.tile([1, C], F32)
        yT_sb = yp.tile([P, 4, NTOK], F16)

        def fc_block(a, cur2T, cur2T8):
            h1T = h1p.tile([P, 16, TCH], FP8 if FC2_FP8 else BF16, tag="h1T")
            for jc in range(16):
                ps = ps_mm.tile([P, TCH], F32, tag="mm")
                # half the contraction (channels 0-255) in fp8 DoubleRow
                t.matmul(ps, lhsT=fc1w8_sb[:, 0:2, ts(jc, P)],
                         rhs=cur2T8[:, 0:2, :],
                         start=True, stop=False, perf_mode=DR)
                for cj in (2, 3):
                    t.matmul(ps, lhsT=fc1w_sb[:, cj, ts(jc, P)], rhs=cur2T[:, cj, :],
                             start=False, stop=(cj == 3))
                s.activation(out=h1T[:, jc, :], in_=ps, func=AF.Gelu,
                             bias=fc1b_sb[:, jc:jc + 1], scale=1.0)
            if a == NT - 1:
                # early pool: sum_ch y[t, ch] == h1[t,:] @ colsum(W2) (+ sum b)
                # lets the gate chain overlap the fc2/y work of this chunk
                for jc in range(16):
                    t.matmul(pool_ps[0:1, :], lhsT=w2s_sb[:, jc:jc + 1],
                             rhs=h1T[:, jc, :],
                             start=False, stop=(jc == 15), skip_group_check=True)
            for cc in range(4):
                ps = ps_mm.tile([P, TCH], F32, tag="mm")
                if FC2_FP8:
                    for k in range(8):
                        t.matmul(ps, lhsT=fc2w_sb[:, 2 * k:2 * k + 2, ts(cc, P)],
                                 rhs=h1T[:, 2 * k:2 * k + 2, :],
                                 start=(k == 0), stop=(k == 7), perf_mode=DR)
                else:
                    for jc in range(16):
                        t.matmul(ps, lhsT=fc2w_sb[:, jc, ts(cc, P)], rhs=h1T[:, jc, :],
                                 start=(jc == 0), stop=(jc == 15))
                yslc = yT_sb[:, cc, ds(a * TCH, TCH)]
                s.activation(out=yslc, in_=ps, func=AF.Identity,
                             bias=fc2b_sb[:, cc:cc + 1],
                             scale=(1.0 / FC2_WSCALE) if FC2_FP8 else 1.0)
                if a < NT - 1:
                    # pooled[i] += sum_ch y[8i+a, ch]  (partition reduction)
                    t.matmul(pool_ps[0:1, :], lhsT=ones_colh, rhs=yslc,
                             start=(a == 0 and cc == 0), stop=False,
                             skip_group_check=True)

        pend2 = None
        for a in range(NT):
            x2ts = []
            for j in range(4):
                ps = ps_mm.tile([P, TCH], F32, tag="mm")
                for cb in range(2):
                    t.matmul(ps, lhsT=qT_sb[:, 2 * cb:2 * cb + 2, ds(a * TCH + j * P, P)],
                             rhs=G_sb[:, 2 * cb:2 * cb + 2, :],
                             start=(cb == 0),
                             stop=(cb == 1 and not flags["proj_bias"]),
                             perf_mode=DR)
                if flags["proj_bias"]:
                    t.matmul(ps, lhsT=ones_row_bf, rhs=projb_sb,
                             start=False, stop=True)
                xt = xin.tile([P, C], F32, tag="xt", bufs=3)
                sy.dma_start(out=xt, in_=x_perm[a][:, j, :])
                x2t = x2p.tile([P, C], BF16, tag="x2t")
                v.tensor_add(out=x2t, in0=ps, in1=xt)
                sy.dma_start(out=x2_perm[a][:, j, :], in_=x2t)
                x2ts.append(x2t)
            curs = ln_dve(x2ts, ln_bc.get("ln2w"), ln_bc.get("ln2b"))
            if pend2 is not None:
                fc_block(*pend2)
            cur2T, cur2T8 = ln_pe(curs, want_fp8=True)
            pend2 = (a, cur2T, cur2T8)
        fc_block(*pend2)

        # ================= TAIL =================
        # ----- ECA gate -----
        ppad = smp.tile([1, C + 2], F32, tag="ppad")
        v.memset(ppad, 0.0)
        s.activation(out=ppad[:, 1:C + 1], in_=pool_ps, func=AF.Identity,
                     bias=fc2bsn_sb[0:1, 0:1], scale=1.0 / NTOK)
        cv = smp.tile([1, C], F32, tag="cv")
        v.tensor_scalar_mul(out=cv, in0=ppad[0:1, 0:C], scalar1=eca_sb[0:1, 0:1])
        v.scalar_tensor_tensor(out=cv, in0=ppad[0:1, 1:C + 1], scalar=eca_sb[0:1, 1:2],
                               in1=cv, op0=ALU.mult, op1=ALU.add)
        v.scalar_tensor_tensor(out=cv, in0=ppad[0:1, 2:C + 2], scalar=eca_sb[0:1, 2:3],
                               in1=cv, op0=ALU.mult, op1=ALU.add)
        s.activation(out=cv, in_=cv, func=AF.Sigmoid)
        cvb = smp.tile([1, C], BF16, tag="cvb")
        s.add(out=cvb, in_=cv, add=1.0)
        psb = ps_mm.tile([P, C], F32, tag="mm")
        t.matmul(psb, lhsT=ones_row, rhs=cvb, start=True, stop=True)
        sB = consts.tile([P, C], BF16)
        v.tensor_copy(out=sB, in_=psb)

        # ----- out[512a+.] += sB * yT_a  (out already holds x2) -----
        # two parallel streams: gpsimd accum-DMA (blocks 0-3) and sync-queue
        # readback + DVE add (blocks 4-7)
        xzs = {}
        for a in range(4, NT):
            xz = outp.tile([P, 4, C], BF16, name=f"xz{a}", tag="xz", bufs=2)
            sy.dma_start(out=xz,
                         in_=out_d[ds(a * TCH, TCH), :].rearrange(
                             "(cc p) c -> p cc c", p=P))
            xzs[a] = xz
        for a in range(NT):
            w = outp.tile([P, 4, C], BF16, tag="w", bufs=3)
            for cc in range(4):
                v.tensor_mul(out=w[:, cc, :],
                             in0=yT_sb[:, cc, ds(a * TCH, TCH)], in1=sB)
            if a < 4:
                g.dma_start(
                    out=out_d[ds(a * TCH, TCH), :].rearrange("(cc p) c -> p cc c", p=P),
                    in_=w, accum_op=ALU.add)
            else:
                for cc in range(4):
                    v.tensor_add(out=w[:, cc, :], in0=w[:, cc, :],
                                 in1=xzs[a][:, cc, :])
                sy.dma_start(
                    out=out_d[ds(a * TCH, TCH), :].rearrange("(cc p) c -> p cc c", p=P),
                    in_=w)

    nc.compile()
    return nc


_CACHE = {}


def _get_program(flags):
    key = tuple(sorted(flags.items()))
    if key not in _CACHE:
        _CACHE[key] = _build(flags)
    return _CACHE[key]


def _host_prep(inputs):
    bf = ml_dtypes.bfloat16
    qkv_w = np.asarray(inputs["qkv_w"], np.float32).copy()
    qkv_w[C:2 * C, :] *= SCALE  # fold attention scale into k weights
    flags = {
        "ln1w": not np.all(inputs["ln1_w"] == 1.0),
        "ln1b": bool(np.any(inputs["ln1_b"] != 0.0)),
        "ln2w": not np.all(inputs["ln2_w"] == 1.0),
        "ln2b": bool(np.any(inputs["ln2_b"] != 0.0)),
        "proj_bias": bool(np.any(inputs["proj_b"] != 0.0)),
    }
    fc2wT = np.ascontiguousarray(np.asarray(inputs["fc2_w"], np.float32).T)
    if FC2_FP8:
        fc2wT = (fc2wT * FC2_WSCALE).astype(ml_dtypes.float8_e4m3)
    else:
        fc2wT = fc2wT.astype(bf)
    common = {
        "wqkvT": np.ascontiguousarray(qkv_w.T).astype(bf),
        "projwT": np.ascontiguousarray(np.asarray(inputs["proj_w"], np.float32).T).astype(bf),
        "fc1wT": np.ascontiguousarray(np.asarray(inputs["fc1_w"], np.float32).T).astype(bf),
        "fc1w8T": np.ascontiguousarray(np.asarray(inputs["fc1_w"], np.float32).T[:2 * P, :])
            .astype(ml_dtypes.float8_e4m3),
        "fc2wT": fc2wT,
        "fc1b": np.ascontiguousarray(
            np.asarray(inputs["fc1_b"], np.float32).reshape(HID // P, P).T),
        "fc2b": np.ascontiguousarray(
            np.asarray(inputs["fc2_b"], np.float32).reshape(C // P, P).T),
        "ecaw": np.asarray(inputs["eca_w"], np.float32).reshape(1, 3),
        "w2s": np.ascontiguousarray(
            np.asarray(inputs["fc2_w"], np.float32).sum(axis=0)
            .reshape(HID // P, P).T)
            .astype(ml_dtypes.float8_e4m3 if FC2_FP8 else bf),
        "fc2bsn": np.asarray(inputs["fc2_b"], np.float32).sum()
            .reshape(1, 1) / NTOK,
    }
    if flags["proj_bias"]:
        common["projb"] = np.asarray(inputs["proj_b"], np.float32).reshape(1, C).astype(bf)
    for nm, key in (("ln1w", "ln1_w"), ("ln1b", "ln1_b"),
                    ("ln2w", "ln2_w"), ("ln2b", "ln2_b")):
        if flags[nm]:
            common[nm] = np.asarray(inputs[key], np.float32)
    return flags, common


def kernel(**inputs):
    flags, common = _host_prep(inputs)
    nc = _get_program(flags)
    x = np.asarray(inputs["x"], np.float32)
    in_maps = [dict(common, x=np.ascontiguousarray(x[i])) for i in range(B)]
    res = run_bass_kernel_spmd(nc, in_maps, list(range(B)))
    return np.stack([np.asarray(r["out"], np.float32) for r in res.results], axis=0)


# revision 36
# speedup vs baseline: 1.0005x; 1.0005x over previous
"""ChannelBlock (XCiT-style cross-covariance attention + MLP w/ ECA gate) on 8 TRN2 cores.

Sharding: data-parallel over batch B=8 (1 batch element per core); all params
replicated.  Per-core problem: x (4096, 512) fp32.  HW ~370-374us (baseline 514us).

Design notes:
  - attn-apply folded into proj: per head G_h[e,c] = sum_d attn_h[d,e]*projW[c,64h+d],
    so x2 = q @ G in one fp8-DoubleRow GEMM chain (no separate apply pass).
  - phase-2/3 token chunks are PERMUTED (chunk a = tokens {8i+a}): the
    reference's .view(B,C,H,W) channel shuffle maps fc2's channel-major output
    tile for chunk a DIRECTLY onto output rows [512a, 512a+512) -- no permute
    pass.  qT is written in permuted order during phase 1 (strided ACT copy).
  - fp8-e4m3 DoubleRow (2 contraction rows/cycle) on fc2 (gelu emits fp8 h1T;
    weights host-quantized at 8x, undone by the y-copy ACT scale), on the
    x2 = q @ G GEMM, and on HALF of fc1's contraction (channels 0-255 as one
    DR-MM + channels 256-511 as two bf16 MMs: error scales with sqrt of the
    quantized fraction).  qkv/logits and the other fc1 half stay bf16:
    measured rel_l2 1.771e-2 vs the 2e-2 gate (full-fc1 fp8 would be 2.03e-2).
  - transposes via NORMAL matmul with identity as the moving operand
    (out = curs.T @ I): pipelines at ~90ns vs ~390ns for transpose-mode.
  - ECA pool via ones-vector matmuls on fc2 output tiles (PE, partition
    reduction); for the LAST chunk pooled comes from h1 @ colsum(fc2_w) right
    after gelu, so the gate chain overlaps the final fc2/y work.
  - out_d (bf16) doubles as the x2 scratch: phase-2/3 writes x2 rows strided;
    the tail adds gated y via two parallel streams (gpsimd accum-DMA for
    blocks 0-3, sync-queue readback + DVE add for 4-7).  Host casts to f32.
  - startup: x chunk 0 split across sy/s queues, chunk 1 across s/g, LN1
    pipelined 2 chunks ahead of the PE; phase-2/3 weights prefetched at nt==2
    on sy/g so the ACT queue keeps feeding PSUM-evacuation copies.
"""

import numpy as np
import ml_dtypes
from contextlib import ExitStack

import concourse.bacc as bacc
import concourse.bass as bass
import concourse.mybir as mybir
import concourse.tile as tile
from concourse.bass import ts, ds
from concourse.bass_utils import run_bass_kernel_spmd
from concourse.masks import make_identity

F32 = mybir.dt.float32
BF16 = mybir.dt.bfloat16
F16 = mybir.dt.float16
FP8 = mybir.dt.float8e4
DR = mybir.MatmulPerfMode.DoubleRow
AF = mybir.ActivationFunctionType
ALU = mybir.AluOpType
AX = mybir.AxisListType

B = 8
NTOK = 4096
C = 512
NH = 8
HD = 64
HID = 2048
NT = 8           # token chunks of 512
TCH = NTOK // NT  # 512 tokens per chunk
P = 128
LN_EPS = 1e-5
SCALE = HD ** -0.5
FC2_WSCALE = 8.0   # fc2 weights stored *8 in fp8; undone by ACT scale
FC2_FP8 = True


def _build(flags):
    """Build the per-core bass program. flags: dict of adaptive bools."""
    nc = bacc.Bacc("TRN2", target_bir_lowering=False, debug=False, num_devices=B)

    x_d = nc.dram_tensor("x", (NTOK, C), F32, kind="ExternalInput").ap()
    wqkvT_d = nc.dram_tensor("wqkvT", (C, 3 * C), BF16, kind="ExternalInput").ap()
    projwT_d = nc.dram_tensor("projwT", (C, C), BF16, kind="ExternalInput").ap()
    fc1wT_d = nc.dram_tensor("fc1wT", (C, HID), BF16, kind="ExternalInput").ap()
    fc1w8T_d = nc.dram_tensor("fc1w8T", (2 * P, HID), FP8, kind="ExternalInput").ap()
    if FC2_FP8:
        fc2wT_d = nc.dram_tensor("fc2wT", (HID, C), FP8, kind="ExternalInput").ap()
    else:
        fc2wT_d = nc.dram_tensor("fc2wT", (HID, C), BF16, kind="ExternalInput").ap()
    fc1b_d = nc.dram_tensor("fc1b", (P, HID // P), F32, kind="ExternalInput").ap()
    fc2b_d = nc.dram_tensor("fc2b", (P, C // P), F32, kind="ExternalInput").ap()
    ecaw_d = nc.dram_tensor("ecaw", (1, 3), F32, kind="ExternalInput").ap()
    w2s_d = nc.dram_tensor("w2s", (P, HID // P), FP8 if FC2_FP8 else BF16,
                           kind="ExternalInput").ap()
    fc2bsn_d = nc.dram_tensor("fc2bsn", (1, 1), F32, kind="ExternalInput").ap()
    if flags["proj_bias"]:
        projb_d = nc.dram_tensor("projb", (1, C), BF16, kind="ExternalInput").ap()
    ln_d = {}
    for nm in ("ln1w", "ln1b", "ln2w", "ln2b"):
        if flags[nm]:
            ln_d[nm] = nc.dram_tensor(nm, (C,), F32, kind="ExternalInput").ap()

    out_d = nc.dram_tensor("out", (NTOK, C), BF16, kind="ExternalOutput").ap()

    v = nc.vector
    g = nc.gpsimd
    s = nc.scalar
    t = nc.tensor
    sy = nc.sync

    # strided views: token n = 1024*j + 8*p + a  <->  [a][p, j, :]
    x_perm = x_d.rearrange("(j p e) c -> e p j c", e=8, p=P)
    x2_perm = out_d.rearrange("(j p e) c -> e p j c", e=8, p=P)

    with tile.TileContext(nc) as tc, ExitStack() as ctx:
        # ---------------- pools ----------------
        consts = ctx.enter_context(tc.tile_pool(name="consts", bufs=1))
        wpool = ctx.enter_context(tc.tile_pool(name="wpool", bufs=1))
        qpool = ctx.enter_context(tc.tile_pool(name="qpool", bufs=1))
        xin = ctx.enter_context(tc.tile_pool(name="xin", bufs=4))
        curp = ctx.enter_context(tc.tile_pool(name="curp", bufs=5))
        curTp = ctx.enter_context(tc.tile_pool(name="curTp", bufs=2))
        kvp = ctx.enter_context(tc.tile_pool(name="kvp", bufs=4))
        statp = ctx.enter_context(tc.tile_pool(name="statp", bufs=3))
        smp = ctx.enter_context(tc.tile_pool(name="smp", bufs=1))
        x2p = ctx.enter_context(tc.tile_pool(name="x2p", bufs=4))
        h1p = ctx.enter_context(tc.tile_pool(name="h1p", bufs=1))
        yp = ctx.enter_context(tc.tile_pool(name="yp", bufs=1))
        outp = ctx.enter_context(tc.tile_pool(name="outp", bufs=2))

        ps_t = ctx.enter_context(tc.tile_pool(name="ps_t", bufs=2, space="PSUM"))
        ps_mm = ctx.enter_context(tc.tile_pool(name="ps_mm", bufs=3, space="PSUM"))
        ps_log = ctx.enter_context(tc.tile_pool(name="ps_log", bufs=1, space="PSUM"))
        ps_pool = ctx.enter_context(tc.tile_pool(name="ps_pool", bufs=1, space="PSUM"))

        # ---------------- phase-1 critical DMAs first ----------------
        # x chunk 0 split into 4 so LN1 starts on first-quarter arrival;
        # chunks 1-2 prefetched immediately after.  Weights for phase 1
        # (wqkv) ride the scalar queue in parallel; everything else waits.
        xbs = {}
        xbs[0] = xin.tile([P, 4, C], F32, name="xb0", tag="xb", bufs=3)
        for q4 in range(4):
            (sy if q4 < 2 else s).dma_start(out=xbs[0][:, q4, :],
                                            in_=x_d[ds(q4 * P, P), :])
        wqkv_sb = wpool.tile([P, 4, 3 * C], BF16)
        for cj in range(4):
            (s if cj % 2 else g).dma_start(out=wqkv_sb[:, cj, :],
                                           in_=wqkvT_d[ts(cj, P), :])
        ident = consts.tile([P, P], BF16)
        make_identity(nc, ident)
        ones_colh = consts.tile([P, 1], F16)   # lhsT for partition-sum of y tiles
        v.memset(ones_colh, 1.0)
        ones_row = consts.tile([1, P], BF16)   # lhsT for broadcast outer product
        v.memset(ones_row, 1.0)
        if flags["proj_bias"]:
            ones_row_bf = consts.tile([1, P], BF16)
            v.memset(ones_row_bf, 1.0)
        # preload the Exp act-table off the critical path (softmax boundary)
        tiny = consts.tile([1, 1], F32)
        v.memset(tiny, 0.0)
        s.activation(out=tiny, in_=tiny, func=AF.Exp)

        ln_bc = {}
        for nm in ln_d:
            bc = wpool.tile([P, C], F32, tag=f"lnbc_{nm}")
            g.dma_start(
                out=bc,
                in_=bass.AP(tensor=ln_d[nm].tensor, offset=ln_d[nm].offset,
                            ap=[[0, P], [1, C]]),
            )
            ln_bc[nm] = bc

        qT_sb = qpool.tile([P, 4, NTOK], FP8)

        def ln_dve(src_tiles, w_bc, b_bc, apply_eng="v"):
            """LayerNorm stats+apply on DVE only -> 4 bf16 cur tiles."""
            mv = statp.tile([P, 4, 2], F32, tag="mv")
            st = statp.tile([P, 6], F32, tag="st6")
            for p in range(4):
                v.bn_stats(out=st, in_=src_tiles[p])
                v.bn_aggr(out=mv[:, p, :], in_=st)
                st = statp.tile([P, 6], F32, tag="st6")
            # rstd = 1/sqrt(var+eps) via DVE reciprocal + 3 Newton steps
            # (avoids ScalarE Sqrt table load between Exp/Gelu phases)
            aN = statp.tile([P, 4], F32, tag="veps")
            v.tensor_scalar_add(out=aN, in0=mv[:, :, 1], scalar1=LN_EPS)
            rstd = statp.tile([P, 4], F32, tag="rstd")
            v.reciprocal(out=rstd, in_=aN)
            tN = statp.tile([P, 4], F32, tag="tN")
            uN = statp.tile([P, 4], F32, tag="uN")
            for _ in range(2):
                v.tensor_mul(out=tN, in0=rstd, in1=rstd)
                v.tensor_mul(out=tN, in0=tN, in1=aN)
                v.tensor_scalar(out=uN, in0=tN, scalar1=-0.5, scalar2=1.5,
                                op0=ALU.mult, op1=ALU.add)
                v.tensor_mul(out=rstd, in0=rstd, in1=uN)
            nmr = None
            if apply_eng == "s":
                nmr = statp.tile([P, 4], F32, tag="nmr")
                v.tensor_mul(out=nmr, in0=mv[:, :, 0], in1=rstd)
                v.tensor_scalar_mul(out=nmr, in0=nmr, scalar1=-1.0)
            curs = []
            for p in range(4):
                if w_bc is None and b_bc is None:
                    cur = curp.tile([P, TCH], BF16, tag="cur", bufs=12)
                    if apply_eng == "s":
                        s.activation(out=cur, in_=src_tiles[p], func=AF.Identity,
                                     bias=nmr[:, p:p + 1], scale=rstd[:, p:p + 1])
                    else:
                        v.tensor_scalar(out=cur, in0=src_tiles[p],
                                        scalar1=mv[:, p, 0:1], scalar2=rstd[:, p:p + 1],
                                        op0=ALU.subtract, op1=ALU.mult)
                else:
                    tmp = curp.tile([P, TCH], F32, tag="curf")
                    v.tensor_scalar(out=tmp, in0=src_tiles[p],
                                    scalar1=mv[:, p, 0:1], scalar2=rstd[:, p:p + 1],
                                    op0=ALU.subtract, op1=ALU.mult)
                    cur = curp.tile([P, TCH], BF16, tag="cur", bufs=12)
                    if w_bc is not None and b_bc is not None:
                        v.tensor_mul(out=tmp, in0=tmp, in1=w_bc)
                        v.tensor_add(out=cur, in0=tmp, in1=b_bc)
                    elif w_bc is not None:
                        v.tensor_mul(out=cur, in0=tmp, in1=w_bc)
                    else:
                        v.tensor_add(out=cur, in0=tmp, in1=b_bc)
                curs.append(cur)
            return curs

        def ln_pe(curs, want_fp8=False):
            """Transpose 4 cur tiles -> channel-major curT [128,4,512].

            Uses normal matmuls with identity as the MOVING operand
            (out = curs_slice.T @ I): pipelines at ~90ns/MM vs ~390ns
            for transpose-mode (which can't overlap fill/drain)."""
            curT = curTp.tile([P, 4, TCH], BF16, tag="curT")
            curT8 = None
            if want_fp8:
                curT8 = curTp.tile([P, 2, TCH], FP8, tag="curT8")
            for cj in range(4):
                pst = ps_t.tile([P, TCH], F32, tag="pst")
                for p in range(4):
                    t.matmul(pst[:, ts(p, P)], lhsT=curs[p][:, ts(cj, P)],
                             rhs=ident, start=True, stop=True)
                if want_fp8 and cj < 2:
                    # fc1's bf16 MMs only read blocks 2-3; blocks 0-1 go fp8
                    v.tensor_copy(out=curT8[:, cj, :], in_=pst)
                else:
                    s.copy(out=curT[:, cj, :], in_=pst)
            return (curT, curT8) if want_fp8 else curT

        # ================= PHASE 1: LN1 + qkv + logits (sw-pipelined) ======
        logits_ps = ps_log.tile([P, 4, P], F32)
        qT_v = qT_sb.rearrange("p jc (a i) -> p jc a i", a=8)

        def pe_block1(nt, curs):
            curT = ln_pe(curs)
            kvts = []
            for p in range(4):
                kvt = kvp.tile([P, 2 * C], BF16, tag="kv")
                for h2 in range(2):
                    ps = ps_mm.tile([P, TCH], F32, tag="mm")
                    for cj in range(4):
                        t.matmul(ps, lhsT=curT[:, cj, ts(p, P)],
                                 rhs=wqkv_sb[:, cj, ds(C + h2 * C, C)],
                                 start=(cj == 0), stop=(cj == 3))
                    (v.tensor_copy if h2 == 0 else s.copy)(
                        out=kvt[:, ts(h2, C)], in_=ps)
                kvts.append(kvt)

            def do_q():
                for jc in range(4):
                    ps = ps_mm.tile([P, TCH], F32, tag="mm")
                    for cj in range(4):
                        t.matmul(ps, lhsT=wqkv_sb[:, cj, ts(jc, P)],
                                 rhs=curT[:, cj, :],
                                 start=(cj == 0), stop=(cj == 3))
                    # permuted write: qT[:, jc, a*512 + 64*nt + i'] = q[8i'+a]
                    s.copy(out=qT_v[:, jc, :, ds(64 * nt, 64)],
                           in_=ps.rearrange("p (i a) -> p a i", a=8))

            def do_logits():
                for p in range(4):
                    for hp in range(4):
                        t.matmul(logits_ps[:, hp, :],
                                 lhsT=kvts[p][:, ds(hp * P, P)],
                                 rhs=kvts[p][:, ds(C + hp * P, P)],
                                 start=(nt == 0 and p == 0 and hp == 0),
                                 stop=(nt == NT - 1 and p == 3 and hp == 3),
                                 skip_group_check=True)

            if nt == NT - 1:
                do_logits()
                do_q()
            else:
                do_q()
                do_logits()

        pends = []
        for nt in range(NT):
            xb = xbs.pop(nt)
            xts = [xb[:, q, :] for q in range(4)]
            curs = ln_dve(xts, ln_bc.get("ln1w"), ln_bc.get("ln1b"))
            # prefetch AFTER ln_dve so the scheduler orders this chunk's
            # LN apply ahead of the next chunks' DMA-dependent stats
            pres = (1, 2, 3) if nt == 0 else ((nt + 3,) if nt + 3 < NT else ())
            for pre in pres:
                xbs[pre] = xin.tile([P, 4, C], F32, name=f"xb{nt}_{pre}", tag="xb", bufs=3)
                if pre == 1:
                    for q4 in range(4):
                        (s if q4 % 2 else g).dma_start(
                            out=xbs[pre][:, q4, :],
                            in_=x_d[ds(pre * TCH + q4 * P, P), :])
                else:
                    sy.dma_start(
                        out=xbs[pre],
                        in_=x_d[ds(pre * TCH, TCH), :].rearrange(
                            "(q p) c -> p q c", p=P))
            pends.append((nt, curs))
            if len(pends) > 2:
                pe_block1(*pends.pop(0))
            if nt == 2:
                # prefetch phase-2/3 weights (sync/gpsimd queues: keeps the
                # ACT queue free for the copy stream)
                projw_sb = wpool.tile([P, 4, C], BF16)
                for dc in range(4):
                    sy.dma_start(out=projw_sb[:, dc, :], in_=projwT_d[ts(dc, P), :])
                fc1w_sb = wpool.tile([P, 4, HID], BF16)
                for cj in range(4):
                    sy.dma_start(out=fc1w_sb[:, cj, :], in_=fc1wT_d[ts(cj, P), :])
                fc1w8_sb = wpool.tile([P, 2, HID], FP8)
                sy.dma_start(out=fc1w8_sb,
                             in_=fc1w8T_d[:, :].rearrange("(q p) c -> p q c", p=P))
                fc2w_sb = wpool.tile([P, 16, C], FP8 if FC2_FP8 else BF16)
                for jc in range(4):
                    g.dma_start(out=fc2w_sb[:, ts(jc, 4), :],
                                in_=fc2wT_d[ds(jc * 4 * P, 4 * P), :].rearrange(
                                    "(q p) c -> p q c", p=P))
                fc1b_sb = wpool.tile([P, HID // P], F32)
                sy.dma_start(out=fc1b_sb, in_=fc1b_d[:, :])
                fc2b_sb = wpool.tile([P, C // P], F32)
                sy.dma_start(out=fc2b_sb, in_=fc2b_d[:, :])
                eca_sb = wpool.tile([1, 3], F32)
                g.dma_start(out=eca_sb, in_=ecaw_d[:, :])
                w2s_sb = wpool.tile([P, HID // P], FP8 if FC2_FP8 else BF16)
                g.dma_start(out=w2s_sb, in_=w2s_d[:, :])
                fc2bsn_sb = wpool.tile([1, 1], F32)
                g.dma_start(out=fc2bsn_sb, in_=fc2bsn_d[:, :])
                if flags["proj_bias"]:
                    projb_sb = wpool.tile([1, C], BF16)
                    g.dma_start(out=projb_sb, in_=projb_d[:, :])
        for pnd in pends:
            pe_block1(*pnd)

        # ================= softmax + G = blockdiag(attn)-contracted projW ====
        G_sb = wpool.tile([P, 4, C], FP8)
        for hp in range(4):
            a128 = smp.tile([P, P], BF16, tag="a128", bufs=2)
            for half in range(2):
                rows = slice(64 * half, 64 * half + 64)
                nm = smp.tile([P, 1], F32, tag="nm", bufs=2)
                v.tensor_reduce(out=nm[rows, :], in_=logits_ps[rows, hp, ds(64 * half, 64)],
                                axis=AX.X, op=ALU.max, negate=True)
                esb = smp.tile([P, 64], F32, tag="esb", bufs=2)
                ssum = smp.tile([P, 1], F32, tag="ssum", bufs=2)
                s.activation(out=esb[rows, :], in_=logits_ps[rows, hp, ds(64 * half, 64)],
                             func=AF.Exp, bias=nm[rows, :], scale=1.0,
                             accum_out=ssum[rows, :])
                v.reciprocal(out=ssum[rows, :], in_=ssum[rows, :])
                v.tensor_scalar_mul(out=a128[rows, ds(64 * half, 64)],
                                    in0=esb[rows, :], scalar1=ssum[rows, :])
            # G_h[e, c] = sum_d attn_h[d, e] * projwT[64h+d, c]
            gps = ps_mm.tile([P, C], F32, tag="mm")
            for half in range(2):
                rows = slice(64 * half, 64 * half + 64)
                t.matmul(gps[rows, :], lhsT=a128[rows, rows],
                         rhs=projw_sb[rows, hp, :], start=True, stop=True)
            s.copy(out=G_sb[:, hp, :], in_=gps)

        # ================= PHASE 2+3: x2 = q@G (+x), LN2, MLP (permuted) =====
        pool_ps = ps_pool.tile([1, C], F32)
        yT_sb = yp.tile([P, 4, NTOK], F16)

        def fc_block(a, cur2T, cur2T8):
            h1T = h1p.tile([P, 16, TCH], FP8 if FC2_FP8 else BF16, tag="h1T")
            for jc in range(16):
                ps = ps_mm.tile([P, TCH], F32, tag="mm")
                # half the contraction (channels 0-255) in fp8 DoubleRow
                t.matmul(ps, lhsT=fc1w8_sb[:, 0:2, ts(jc, P)],
                         rhs=cur2T8[:, 0:2, :],
                         start=True, stop=False, perf_mode=DR)
                for cj in (2, 3):
                    t.matmul(ps, lhsT=fc1w_sb[:, cj, ts(jc, P)], rhs=cur2T[:, cj, :],
                             start=False, stop=(cj == 3))
                s.activation(out=h1T[:, jc, :], in_=ps, func=AF.Gelu,
                             bias=fc1b_sb[:, jc:jc + 1], scale=1.0)
            if a == NT - 1:
                # early pool: sum_ch y[t, ch] == h1[t,:] @ colsum(W2) (+ sum b)
                # lets the gate chain overlap the fc2/y work of this chunk
                for jc in range(16):
                    t.matmul(pool_ps[0:1, :], lhsT=w2s_sb[:, jc:jc + 1],
                             rhs=h1T[:, jc, :],
                             start=False, stop=(jc == 15), skip_group_check=True)
            for cc in range(4):
                ps = ps_mm.tile([P, TCH], F32, tag="mm")
                if FC2_FP8:
                    for k in range(8):
                        t.matmul(ps, lhsT=fc2w_sb[:, 2 * k:2 * k + 2, ts(cc, P)],
                                 rhs=h1T[:, 2 * k:2 * k + 2, :],
                                 start=(k == 0), stop=(k == 7), perf_mode=DR)
                else:
                    for jc in range(16):
                        t.matmul(ps, lhsT=fc2w_sb[:, jc, ts(cc, P)], rhs=h1T[:, jc, :],
                                 start=(jc == 0), stop=(jc == 15))
                yslc = yT_sb[:, cc, ds(a * TCH, TCH)]
                s.activation(out=yslc, in_=ps, func=AF.Identity,
                             bias=fc2b_sb[:, cc:cc + 1],
                             scale=(1.0 / FC2_WSCALE) if FC2_FP8 else 1.0)
                if a < NT - 1:
                    # pooled[i] += sum_ch y[8i+a, ch]  (partition reduction)
                    t.matmul(pool_ps[0:1, :], lhsT=ones_colh, rhs=yslc,
                             start=(a == 0 and cc == 0), stop=False,
                             skip_group_check=True)

        pend2 = None
        for a in range(NT):
            x2ts = []
            for j in range(4):
                ps = ps_mm.tile([P, TCH], F32, tag="mm")
                for cb in range(2):
                    t.matmul(ps, lhsT=qT_sb[:, 2 * cb:2 * cb + 2, ds(a * TCH + j * P, P)],
                             rhs=G_sb[:, 2 * cb:2 * cb + 2, :],
                             start=(cb == 0),
                             stop=(cb == 1 and not flags["proj_bias"]),
                             perf_mode=DR)
                if flags["proj_bias"]:
                    t.matmul(ps, lhsT=ones_row_bf, rhs=projb_sb,
                             start=False, stop=True)
                xt = xin.tile([P, C], F32, tag="xt", bufs=3)
                sy.dma_start(out=xt, in_=x_perm[a][:, j, :])
                x2t = x2p.tile([P, C], BF16, tag="x2t")
                v.tensor_add(out=x2t, in0=ps, in1=xt)
                sy.dma_start(out=x2_perm[a][:, j, :], in_=x2t)
                x2ts.append(x2t)
            curs = ln_dve(x2ts, ln_bc.get("ln2w"), ln_bc.get("ln2b"))
            if pend2 is not None:
                fc_block(*pend2)
            cur2T, cur2T8 = ln_pe(curs, want_fp8=True)
            pend2 = (a, cur2T, cur2T8)
        fc_block(*pend2)

        # ================= TAIL =================
        # ----- ECA gate -----
        ppad = smp.tile([1, C + 2], F32, tag="ppad")
        v.memset(ppad, 0.0)
        s.activation(out=ppad[:, 1:C + 1], in_=pool_ps, func=AF.Identity,
                     bias=fc2bsn_sb[0:1, 0:1], scale=1.0 / NTOK)
        cv = smp.tile([1, C], F32, tag="cv")
        v.tensor_scalar_mul(out=cv, in0=ppad[0:1, 0:C], scalar1=eca_sb[0:1, 0:1])
        v.scalar_tensor_tensor(out=cv, in0=ppad[0:1, 1:C + 1], scalar=eca_sb[0:1, 1:2],
                               in1=cv, op0=ALU.mult, op1=ALU.add)
        v.scalar_tensor_tensor(out=cv, in0=ppad[0:1, 2:C + 2], scalar=eca_sb[0:1, 2:3],
                               in1=cv, op0=ALU.mult, op1=ALU.add)
        s.activation(out=cv, in_=cv, func=AF.Sigmoid)
        cvb = smp.tile([1, C], BF16, tag="cvb")
        s.add(out=cvb, in_=cv, add=1.0)
        psb = ps_mm.tile([P, C], F32, tag="mm")
        t.matmul(psb, lhsT=ones_row, rhs=cvb, start=True, stop=True)
        sB = consts.tile([P, C], BF16)
        v.tensor_copy(out=sB, in_=psb)

        # ----- out[512a+.] += sB * yT_a  (out already holds x2) -----
        # two parallel streams: gpsimd accum-DMA (blocks 0-3) and sync-queue
        # readback + DVE add (blocks 4-7)
        xzs = {}
        for a in range(4, NT):
            xz = outp.tile([P, 4, C], BF16, name=f"xz{a}", tag="xz", bufs=2)
            sy.dma_start(out=xz,
                         in_=out_d[ds(a * TCH, TCH), :].rearrange(
                             "(cc p) c -> p cc c", p=P))
            xzs[a] = xz
        for a in range(NT):
            w = outp.tile([P, 4, C], BF16, tag="w", bufs=3)
            for cc in range(4):
                v.tensor_mul(out=w[:, cc, :],
                             in0=yT_sb[:, cc, ds(a * TCH, TCH)], in1=sB)
            if a < 4:
                g.dma_start(
                    out=out_d[ds(a * TCH, TCH), :].rearrange("(cc p) c -> p cc c", p=P),
                    in_=w, accum_op=ALU.add)
            else:
                for cc in range(4):
                    v.tensor_add(out=w[:, cc, :], in0=w[:, cc, :],
                                 in1=xzs[a][:, cc, :])
                sy.dma_start(
                    out=out_d[ds(a * TCH, TCH), :].rearrange("(cc p) c -> p cc c", p=P),
                    in_=w)

    nc.compile()
    return nc


_CACHE = {}


def _get_program(flags):
    key = tuple(sorted(flags.items()))
    if key not in _CACHE:
        _CACHE[key] = _build(flags)
    return _CACHE[key]


def _host_prep(inputs):
    bf = ml_dtypes.bfloat16
    qkv_w = np.asarray(inputs["qkv_w"], np.float32).copy()
    qkv_w[C:2 * C, :] *= SCALE  # fold attention scale into k weights
    flags = {
        "ln1w": not np.all(inputs["ln1_w"] == 1.0),
        "ln1b": bool(np.any(inputs["ln1_b"] != 0.0)),
        "ln2w": not np.all(inputs["ln2_w"] == 1.0),
        "ln2b": bool(np.any(inputs["ln2_b"] != 0.0)),
        "proj_bias": bool(np.any(inputs["proj_b"] != 0.0)),
    }
    fc2wT = np.ascontiguousarray(np.asarray(inputs["fc2_w"], np.float32).T)
    if FC2_FP8:
        fc2wT = (fc2wT * FC2_WSCALE).astype(ml_dtypes.float8_e4m3)
    else:
        fc2wT = fc2wT.astype(bf)
    common = {
        "wqkvT": np.ascontiguousarray(qkv_w.T).astype(bf),
        "projwT": np.ascontiguousarray(np.asarray(inputs["proj_w"], np.float32).T).astype(bf),
        "fc1wT": np.ascontiguousarray(np.asarray(inputs["fc1_w"], np.float32).T).astype(bf),
        "fc1w8T": np.ascontiguousarray(np.asarray(inputs["fc1_w"], np.float32).T[:2 * P, :])
            .astype(ml_dtypes.float8_e4m3),
        "fc2wT": fc2wT,
        "fc1b": np.ascontiguousarray(
            np.asarray(inputs["fc1_b"], np.float32).reshape(HID // P, P).T),
        "fc2b": np.ascontiguousarray(
            np.asarray(inputs["fc2_b"], np.float32).reshape(C // P, P).T),
        "ecaw": np.asarray(inputs["eca_w"], np.float32).reshape(1, 3),
        "w2s": np.ascontiguousarray(
            np.asarray(inputs["fc2_w"], np.float32).sum(axis=0)
            .reshape(HID // P, P).T)
            .astype(ml_dtypes.float8_e4m3 if FC2_FP8 else bf),
        "fc2bsn": np.asarray(inputs["fc2_b"], np.float32).sum()
            .reshape(1, 1) / NTOK,
    }
    if flags["proj_bias"]:
        common["projb"] = np.asarray(inputs["proj_b"], np.float32).reshape(1, C).astype(bf)
    for nm, key in (("ln1w", "ln1_w"), ("ln1b", "ln1_b"),
                    ("ln2w", "ln2_w"), ("ln2b", "ln2_b")):
        if flags[nm]:
            common[nm] = np.asarray(inputs[key], np.float32)
    return flags, common


def kernel(**inputs):
    flags, common = _host_prep(inputs)
    nc = _get_program(flags)
    x = np.asarray(inputs["x"], np.float32)
    in_maps = [dict(common, x=np.ascontiguousarray(x[i])) for i in range(B)]
    res = run_bass_kernel_spmd(nc, in_maps, list(range(B)))
    return np.stack([np.asarray(r["out"], np.float32) for r in res.results], axis=0)
